# revision 1
# baseline (speedup 1.0000x reference)
"""Causal self-attention (B=2, T=2048, C=1024, H=16) on 8 TRN2 NeuronCores.

Sharding: core c -> batch b = c//4, head-group g = c%4 (4 heads = 256 channels).
Each core computes its 4 heads end-to-end and a partial projection
(y_local @ W_proj[256g:256g+256, :]); the host sums the 4 partials per batch.

On-chip dataflow (matmuls fp32r = full-rate fp32, ~1.6e-4 rel err):
  qkT[ch, t]  = Wqkv[:, ch].T @ x[b].T          (q,k kept transposed: d on partitions)
  v[t, ch]    = x[b] @ Wv                       (natural layout, + ones column per head)
  S^T[k, q]   = k_h @ q_h^T  (per head, row-packed 2 heads per PE pass, K=64;
                diagonal chunks narrowed to their causally-valid column window)
  causal mask: short bf16 identity-matmul accumulates -1e30 onto the masked
                prefix of diagonal chunks (keeps mask work off DVE)
  P = exp(S^T) on ScalarE, one [128,1024] op per head pair
  y^T[d, q], denom[q] = [V_h | 1].T @ P        (ones column -> denominator row)
  y_norm^T = y^T * (1/denom)  (gpsimd partition_broadcast + DVE mult)
  out_partial[t, c] = y_norm^T.T @ W_proj_slice

Scheduling: engines execute streams in emission(priority) order, so qkv/proj
work is explicitly interleaved into the ACT-bound attention chunks (filler
queue), the input DMA ramp is filled with split-k first-half passes, and xT
streams in column halves so attention(0) unlocks after 7 of the 12 MB.
"""

import numpy as np

B, T, C = 2, 2048, 1024
H, HD = 16, 64
NCORES = 8
HEADS_PER_CORE = 4          # 2 pairs
CH = HEADS_PER_CORE * HD    # 256 channels per core
KT = C // 128               # 8 contraction tiles for qkv
NT = T // 128               # 16 key tiles / t tiles
NJ = T // 512               # 4 query chunks
SCALE = 1.0 / np.sqrt(HD)

_COMPILED = None  # (nc, names) cache


def _build():
    import concourse.bass as bass
    import concourse.bacc as bacc
    import concourse.mybir as mybir
    import concourse.tile as tile

    f32 = mybir.dt.float32
    f32r = mybir.dt.float32r
    r = lambda ap: ap.bitcast(f32r)

    nc = bacc.Bacc("TRN2", target_bir_lowering=False, debug=False)

    xT_d = nc.dram_tensor("xT", [C, T], f32, kind="ExternalInput").ap()
    wqkv_d = nc.dram_tensor("wqkv", [C, 3 * CH], f32, kind="ExternalInput").ap()
    bqk_d = nc.dram_tensor("bqk", [128, 4], f32, kind="ExternalInput").ap()
    bv_d = nc.dram_tensor("bv", [1, CH], f32, kind="ExternalInput").ap()
    wproj_d = nc.dram_tensor("wproj", [CH, C], f32, kind="ExternalInput").ap()
    bf16 = mybir.dt.bfloat16
    mask_d = nc.dram_tensor("mask", [128, 4 * 512], bf16, kind="ExternalInput").ap()
    ident_d = nc.dram_tensor("ident", [128, 128], bf16, kind="ExternalInput").ap()
    out_d = nc.dram_tensor("out_p", [T, C], f32, kind="ExternalOutput").ap()

    with tile.TileContext(nc) as tc:
        with (
            tc.tile_pool(name="p_w", bufs=1) as p_w,
            tc.tile_pool(name="p_x", bufs=1) as p_x,
            tc.tile_pool(name="p_qk", bufs=1) as p_qk,
            tc.tile_pool(name="p_v", bufs=1) as p_v,
            tc.tile_pool(name="p_y", bufs=1) as p_y,
            tc.tile_pool(name="p_p", bufs=3) as p_p,
            tc.tile_pool(name="p_sm", bufs=2) as p_sm,
            tc.tile_pool(name="ps_mm", bufs=2, space="PSUM") as ps_mm,
            tc.tile_pool(name="ps_s", bufs=4, space="PSUM") as ps_s,
            tc.tile_pool(name="ps_y", bufs=2, space="PSUM") as ps_y,
        ):
            # ---- persistent inputs -------------------------------------
            wqkv = [p_w.tile([128, 3 * CH], f32r, name=f"wqkv{k}", tag=f"wqkv{k}")
                    for k in range(KT)]
            xT = [p_x.tile([128, T], f32r, name=f"xT{k}", tag=f"xT{k}")
                  for k in range(KT)]
            wproj = [p_w.tile([128, C], f32r, name=f"wproj{k}", tag=f"wproj{k}")
                     for k in range(2)]
            mask = p_w.tile([128, 4 * 512], bf16, name="mask", tag="mask")
            ident = p_w.tile([128, 128], bf16, name="ident", tag="ident")
            bqk = p_w.tile([128, 4], f32, name="bqk", tag="bqk")
            bvrow = p_w.tile([1, CH], f32, name="bvrow", tag="bvrow")
            bvb = p_w.tile([128, CH], f32, name="bvb", tag="bvb")

            # t-columns 0:1024 of xT unlock waves 0/1 + attention(0);
            # the upper half is only needed by waves 2/3 and streams in later.
            for k in range(KT):
                nc.sync.dma_start(out=wqkv[k], in_=r(wqkv_d[128 * k:128 * (k + 1), :]))
                nc.sync.dma_start(out=xT[k][:, 0:1024],
                                  in_=r(xT_d[128 * k:128 * (k + 1), 0:1024]))
                if k == 1:
                    nc.sync.dma_start(out=mask, in_=mask_d)
                    nc.sync.dma_start(out=ident, in_=ident_d)
                    nc.sync.dma_start(out=bqk, in_=bqk_d)
                    nc.sync.dma_start(out=bvrow, in_=bv_d)
            for k in range(KT):
                nc.sync.dma_start(out=xT[k][:, 1024:2048],
                                  in_=r(xT_d[128 * k:128 * (k + 1), 1024:2048]))
            for k in range(2):
                nc.sync.dma_start(out=wproj[k], in_=r(wproj_d[128 * k:128 * (k + 1), :]))
            nc.gpsimd.partition_broadcast(bvb, bvrow[0:1, :])

            # ---- persistent intermediates ------------------------------
            # qT/kT: [128ch, T]; tile p holds heads (2p, 2p+1) on partitions 0:64/64:128
            qT = [p_qk.tile([128, T], f32r, name=f"qT{p}", tag=f"qT{p}") for p in range(2)]
            kT = [p_qk.tile([128, T], f32r, name=f"kT{p}", tag=f"kT{p}") for p in range(2)]
            # v tiles: [128 t, 4 heads * 65] (65th col of each head = 1.0)
            v = [p_v.tile([128, 4 * 65], f32r, name=f"v{m}", tag=f"v{m}") for m in range(NT)]
            # normalized y^T pair tiles
            yT = [p_y.tile([128, T], f32r, name=f"yT{p}", tag=f"yT{p}") for p in range(2)]

            def qkv_chunk(mi, nj, pool=None, tag=None):
                """qkv^T channels [128mi,128mi+128), t [512nj, 512nj+512)."""
                pool = pool or ps_mm
                ps = pool.tile([128, 512], f32, name="ps_qkv", tag=tag or "mm")
                for k in range(KT):
                    nc.tensor.matmul(
                        ps[:, 0:512],
                        lhsT=r(wqkv[k][:, 128 * mi:128 * (mi + 1)]),
                        rhs=r(xT[k][:, 512 * nj:512 * (nj + 1)]),
                        start=(k == 0), stop=(k == KT - 1),
                    )
                dst = qT[mi] if mi < 2 else kT[mi - 2]
                nc.vector.tensor_scalar_add(
                    dst[:, 512 * nj:512 * (nj + 1)], ps[:, 0:512], bqk[:, mi:mi + 1])

            def v_chunk(m):
                """v rows [128m, 128m+128), all 256 channels, into 65-strided tile."""
                ps = ps_mm.tile([128, 512], f32, name="ps_v", tag="mm")
                for k in range(KT):
                    nc.tensor.matmul(
                        ps[:, 0:CH],
                        lhsT=r(xT[k][:, 128 * m:128 * (m + 1)]),
                        rhs=r(wqkv[k][:, 2 * CH:3 * CH]),
                        start=(k == 0), stop=(k == KT - 1),
                    )
                for h in range(4):
                    nc.vector.memset(v[m][:, 65 * h + 64:65 * h + 65].bitcast(f32), 1.0)
                vi = v[m].rearrange("p (h c) -> p h c", h=4)[:, :, 0:64]
                nc.vector.tensor_tensor(
                    vi,
                    ps[:, 0:CH].rearrange("p (h c) -> p h c", h=4),
                    bvb.rearrange("p (h c) -> p h c", h=4),
                    mybir.AluOpType.add,
                )

            def qkv_chunk_split(mi, nj, half, pool, tag):
                ps = pool.tile([128, 512], f32, name="ps_qkvs", tag=tag)
                for k in range(4 * half, 4 * half + 4):
                    nc.tensor.matmul(
                        ps[:, 0:512],
                        lhsT=r(wqkv[k][:, 128 * mi:128 * (mi + 1)]),
                        rhs=r(xT[k][:, 512 * nj:512 * (nj + 1)]),
                        start=(k % 4 == 0), stop=(k % 4 == 3),
                    )
                dst = (qT[mi] if mi < 2 else kT[mi - 2])[:, 512 * nj:512 * (nj + 1)]
                if half == 0:
                    nc.vector.tensor_scalar_add(dst, ps[:, 0:512], bqk[:, mi:mi + 1])
                else:
                    nc.vector.tensor_tensor(dst, ps[:, 0:512], dst, mybir.AluOpType.add)

            def v_chunk_split(m, half, pool=None, tag=None):
                pool = pool or ps_mm
                ps = pool.tile([128, 512], f32, name="ps_vs", tag=tag or "mm")
                for k in range(4 * half, 4 * half + 4):
                    nc.tensor.matmul(
                        ps[:, 0:CH],
                        lhsT=r(xT[k][:, 128 * m:128 * (m + 1)]),
                        rhs=r(wqkv[k][:, 2 * CH:3 * CH]),
                        start=(k % 4 == 0), stop=(k % 4 == 3),
                    )
                vi = v[m].rearrange("p (h c) -> p h c", h=4)[:, :, 0:64]
                psv = ps[:, 0:CH].rearrange("p (h c) -> p h c", h=4)
                if half == 0:
                    for h in range(4):
                        nc.vector.memset(
                            v[m][:, 65 * h + 64:65 * h + 65].bitcast(f32), 1.0)
                    nc.vector.tensor_tensor(
                        vi, psv, bvb.rearrange("p (h c) -> p h c", h=4),
                        mybir.AluOpType.add)
                else:
                    nc.vector.tensor_tensor(vi, psv, vi, mybir.AluOpType.add)

            def attention(j, p, filler=None):
                """q-chunk j (512 queries), head pair p (heads 2p, 2p+1)."""
                ni = 4 * j + 4  # k-tiles 0..ni-1 are (partially) unmasked
                yA = ps_y.tile([128, 512], f32, name="yA", tag="y")
                yB = ps_y.tile([128, 512], f32, name="yB", tag="y")
                qs = slice(512 * j, 512 * (j + 1))
                for i in range(ni):
                    sA = ps_s.tile([128, 512], f32, name="s_a", tag="s")
                    sB = ps_s.tile([128, 512], f32, name="s_b", tag="s")
                    rr = i - 4 * j
                    diag = rr >= 0
                    # valid window for diagonal chunks: q >= 128*rr + k.
                    # W0 rounded down to 256 keeps the moving dim >= 256
                    # (full-rate fp32r); [W0:512) of each half is computed.
                    W0 = 0 if not diag else min(128 * rr, 256)
                    Wd = 512 - W0
                    qw = slice(512 * j + W0, 512 * (j + 1))
                    # S^T chunks for both heads, row-packed (K=64 each)
                    nc.tensor.matmul(
                        sA[:, W0:512],
                        lhsT=r(kT[p][0:64, 128 * i:128 * (i + 1)]),
                        rhs=r(qT[p][0:64, qw]),
                        start=True, stop=not diag,
                    )
                    nc.tensor.matmul(
                        sB[:, W0:512],
                        lhsT=r(kT[p][64:128, 128 * i:128 * (i + 1)]),
                        rhs=r(qT[p][64:128, qw]),
                        start=True, stop=not diag,
                    )
                    pt = p_p.tile([128, 1024], f32r, name="pt", tag="pt")
                    if diag:
                        # causal mask: short bf16 matmul accumulates -1e30 onto
                        # the masked prefix of the window
                        Wm = 128 * (rr + 1) - W0
                        for half, sh in ((0, sA), (1, sB)):
                            nc.tensor.matmul(
                                sh[:, W0:W0 + Wm],
                                lhsT=ident,
                                rhs=mask[:, 512 * rr:512 * rr + Wm],
                                start=False, stop=True,
                            )
                    for half, sh in ((0, sA), (1, sB)):
                        nc.scalar.activation(
                            pt[:, 512 * half + W0:512 * half + 512],
                            sh[:, W0:512],
                            mybir.ActivationFunctionType.Exp)
                    if filler is not None:
                        filler()
                    nc.tensor.matmul(
                        yA[0:65, W0:512],
                        lhsT=r(v[i][:, 65 * (2 * p):65 * (2 * p) + 65]),
                        rhs=r(pt[:, W0:512]),
                        start=(i == 0), stop=(i == ni - 1),
                    )
                    nc.tensor.matmul(
                        yB[0:65, W0:512],
                        lhsT=r(v[i][:, 65 * (2 * p + 1):65 * (2 * p + 1) + 65]),
                        rhs=r(pt[:, 512 + W0:1024]),
                        start=(i == 0), stop=(i == ni - 1),
                    )
                # normalize: row 64 of y psum = softmax denominator.
                # NB: partition_broadcast reads physical partition 0 on HW
                # (ignores AP partition offset) -> each recip gets its own tile.
                rcA = p_sm.tile([1, 512], f32, name="rcA", tag="rcA")
                rcB = p_sm.tile([1, 512], f32, name="rcB", tag="rcB")
                nc.vector.reciprocal(rcA, yA[64:65, :])
                nc.vector.reciprocal(rcB, yB[64:65, :])
                bcA = p_sm.tile([64, 512], f32, name="bcA", tag="bcA")
                bcB = p_sm.tile([64, 512], f32, name="bcB", tag="bcB")
                nc.gpsimd.partition_broadcast(bcA, rcA[0:1, :])
                nc.gpsimd.partition_broadcast(bcB, rcB[0:1, :])
                nc.vector.tensor_tensor(
                    yT[p][0:64, qs], yA[0:64, :], bcA, mybir.AluOpType.mult)
                nc.vector.tensor_tensor(
                    yT[p][64:128, qs], yB[0:64, :], bcB, mybir.AluOpType.mult)

            def proj(m):
                """output rows [128m, 128m+128)."""
                for u in range(2):
                    if u == 0:
                        ps = ps_s.tile([128, 512], f32, name="ps_pr", tag="s")
                    else:
                        ps = ps_mm.tile([128, 512], f32, name="ps_pr2", tag="mm")
                    for kk in range(2):
                        nc.tensor.matmul(
                            ps[:, 0:512],
                            lhsT=r(yT[kk][:, 128 * m:128 * (m + 1)]),
                            rhs=r(wproj[kk][:, 512 * u:512 * (u + 1)]),
                            start=(kk == 0), stop=(kk == 1),
                        )
                    st = p_p.tile([128, 512], f32, name="st_pr", tag="st_pr", bufs=6)
                    if u == 0:
                        nc.vector.tensor_copy(st, ps[:, 0:512])
                        eng = nc.sync
                    else:
                        nc.scalar.copy(st, ps[:, 0:512])
                        eng = nc.gpsimd
                    eng.dma_start(
                        out=out_d[128 * m:128 * (m + 1), 512 * u:512 * (u + 1)],
                        in_=st)

            # ---- emission order (scheduling priority) -------------------
            # Engines execute their instruction streams in emission (priority)
            # order, so prefetch work must be explicitly interleaved into the
            # ACT-bound attention chunks via a filler queue.
            # ramp: first halves of waves 0+1 run while x4..7 stream in;
            # wave-0 second halves unlock attention(0); wave-1 second halves
            # become the j=0 fillers.
            for nj in (0, 1):
                for mi in (0, 2):
                    qkv_chunk_split(mi, nj, 0, ps_s, "s")
                for m in range(4 * nj, 4 * nj + 4):
                    v_chunk_split(m, 0)
                for mi in (1, 3):
                    qkv_chunk_split(mi, nj, 0, ps_s, "s")
            for mi in (0, 2):
                qkv_chunk_split(mi, 0, 1, ps_s, "s")
            for m in range(4):
                v_chunk_split(m, 1)
            for mi in (1, 3):
                qkv_chunk_split(mi, 0, 1, ps_s, "s")
            for mi in (0, 2, 1, 3):
                qkv_chunk_split(mi, 2, 0, ps_mm, "mm")
            for m in range(8, 12):
                v_chunk_split(m, 0)

            fillers = []

            def filler():
                if fillers:
                    fillers.pop(0)()

            for j in range(NJ):
                if j == 0:  # wave-1 second halves (firsts ran in the ramp)
                    for mi in (0, 2):
                        fillers.append(
                            lambda mi=mi: qkv_chunk_split(mi, 1, 1, ps_mm, "mm"))
                    for m in range(4, 8):
                        fillers.append(lambda m=m: v_chunk_split(m, 1))
                    for mi in (1, 3):
                        fillers.append(
                            lambda mi=mi: qkv_chunk_split(mi, 1, 1, ps_mm, "mm"))
                elif j == 1:  # wave-2 second halves (firsts ran in the ramp)
                    for mi in (0, 2):
                        fillers.append(
                            lambda mi=mi: qkv_chunk_split(mi, 2, 1, ps_mm, "mm"))
                    for m in range(8, 12):
                        fillers.append(lambda m=m: v_chunk_split(m, 1))
                    for mi in (1, 3):
                        fillers.append(
                            lambda mi=mi: qkv_chunk_split(mi, 2, 1, ps_mm, "mm"))
                elif j + 1 < NJ:  # next wave's qkv/v chunks, as fillers
                    for mi in (0, 2, 1, 3):
                        fillers.append(lambda mi=mi, nj=j + 1: qkv_chunk(mi, nj))
                    for m in range(4 * (j + 1), 4 * (j + 2)):
                        fillers.append(lambda m=m: v_chunk(m))
                if j > 0:  # previous chunk's projection: half now, half next j
                    lo = 4 * (j - 1)
                    for m in range(lo, lo + (2 if j < 3 else 4)):
                        fillers.append(lambda m=m: proj(m))
                if j == 3:  # deferred halves of proj(0), proj(1)
                    for m in (2, 3, 6, 7):
                        fillers.append(lambda m=m: proj(m))
                for p in range(2):
                    attention(j, p, filler)
                # drain what the chunks could not absorb before the boundary
                while fillers:
                    fillers.pop(0)()
            for m in range(12, 16):
                proj(m)

    nc.compile()
    return nc


def _host_inputs(x, W_attn, b_attn, W_proj):
    """Build the 8 per-core input maps (numpy only)."""
    x = np.asarray(x, dtype=np.float32)
    W_attn = np.asarray(W_attn, dtype=np.float32)
    b_attn = np.asarray(b_attn, dtype=np.float32)
    W_proj = np.asarray(W_proj, dtype=np.float32)

    import ml_dtypes
    # additive causal masks, windowed: for diag offset r the S chunk is
    # computed on columns [W0, 512) (W0 = min(128r, 256)); the mask pattern at
    # offset 512r covers the masked prefix q' < 128r + k - W0 of that window.
    kl = np.arange(128)[:, None]
    blocks = []
    for rr in range(4):
        W0 = min(128 * rr, 256)
        qp = np.arange(512)[None, :] + W0
        blocks.append(np.where(qp >= kl + 128 * rr, 0.0, -1e30))
    mask = np.concatenate(blocks, axis=1).astype(ml_dtypes.bfloat16)
    ident = np.eye(128, dtype=ml_dtypes.bfloat16)

    in_maps = []
    for c in range(NCORES):
        b, g = divmod(c, 4)
        sl = slice(CH * g, CH * (g + 1))
        wq = W_attn[:, 0 * C:1 * C][:, sl] * SCALE
        wk = W_attn[:, 1 * C:2 * C][:, sl]
        wv = W_attn[:, 2 * C:3 * C][:, sl]
        bq = b_attn[0 * C:1 * C][sl] * SCALE
        bk = b_attn[1 * C:2 * C][sl]
        bv = b_attn[2 * C:3 * C][sl]
        bqk = np.stack([bq[0:128], bq[128:256], bk[0:128], bk[128:256]], axis=1)
        in_maps.append({
            "xT": np.ascontiguousarray(x[b].T),
            "wqkv": np.ascontiguousarray(np.concatenate([wq, wk, wv], axis=1)),
            "bqk": np.ascontiguousarray(bqk),
            "bv": np.ascontiguousarray(bv[None, :]),
            "wproj": np.ascontiguousarray(W_proj[sl, :]),
            "mask": mask,
            "ident": ident,
        })
    return in_maps


def kernel(x, W_attn, b_attn, W_proj, b_proj, _want_results=None):
    global _COMPILED
    from concourse.bass_utils import run_bass_kernel_spmd

    if _COMPILED is None:
        _COMPILED = _build()
    nc = _COMPILED

    in_maps = _host_inputs(x, W_attn, b_attn, W_proj)
    kw = dict(_want_results or {})
    res = run_bass_kernel_spmd(nc, in_maps, core_ids=list(range(NCORES)), **kw)
    if _want_results is not None:
        kernel.last_results = res

    out = np.zeros((B, T, C), dtype=np.float32)
    for c in range(NCORES):
        out[c // 4] += res.results[c]["out_p"]
    out += np.asarray(b_proj, dtype=np.float32)[None, None, :]
    return out



# revision 41
# speedup vs baseline: 1.3561x; 1.3561x over previous
"""Causal self-attention (B=2, T=2048, C=1024, H=16) on 8 TRN2 NeuronCores.

Sharding: core c -> batch b = c//4, head-group g = c%4 (4 heads = 256 channels).
Each core computes its 4 heads end-to-end and a partial projection
(y_local @ W_proj[256g:256g+256, :]); the host sums the 4 partials per batch.

v2 dataflow (cost-model-driven):
  - QKV/V matmuls in fp8e4m3 DoubleRow with two-term compensation:
    (x8+xr8)@(w8+wr8) dropping xr8@wr8 -> 12 DR passes per chunk at
    0.5 cyc/col (0.75x the fp32r cost), weights pre-scaled x64 on host
    (descaled in the psum->sbuf move) to clear fp8's subnormal floor.
  - S^T = k_h^T q_h per head in f16, exact 128-granular causal windows.
    Both heads of a pair share one [128,2,512] psum tile; one fused exp
    per chunk on ACT.
  - Causal masking via 0/1 f16 multiply on DVE (2x mode) after exp --
    no mask matmuls on PE.
  - AV in natural [q,d] layout: lhsT = pt q-slices (stationary loads are
    free in the cost model), rhs = [1|v_h] -> 65-col matmuls, half the
    transposed-layout cost. Softmax denominator rides column 0; the
    normalize is a per-partition tensor_scalar divide (no broadcasts).
  - Normalized y transposed back via PE is_transpose (f16), then f16 proj.
  - Output f16; host upcasts, sums partials, adds b_proj.

Scheduling: engines run their streams in emission order; qkv/v waves and
proj work are interleaved into the ACT-paced attention chunks as fillers.
Outputs DMA via gpsimd SWDGE to keep HWDGE free for the input stream.
"""

import numpy as np

B, T, C = 2, 2048, 1024
H, HD = 16, 64
NCORES = 8
HEADS_PER_CORE = 4          # 2 pairs
CH = HEADS_PER_CORE * HD    # 256 channels per core
KT8 = 4                     # fp8 contraction pair-tiles (K=256 each)
NT = T // 128               # 16 t tiles
NJ = T // 512               # 4 query chunks
SCALE = 1.0 / np.sqrt(HD)
WS = 64.0                   # host-side weight pre-scale for fp8 range

_COMPILED = None


def _build():
    import concourse.bass as bass
    import concourse.bacc as bacc
    import concourse.mybir as mybir
    import concourse.tile as tile

    f32 = mybir.dt.float32
    f16 = mybir.dt.float16
    f8 = mybir.dt.float8e4
    DR = mybir.MatmulPerfMode.DoubleRow
    Exp = mybir.ActivationFunctionType.Exp
    mult = mybir.AluOpType.mult
    add = mybir.AluOpType.add
    div = mybir.AluOpType.divide

    nc = bacc.Bacc("TRN2", target_bir_lowering=False, debug=False)

    # combined main||residual fp8 tensors: one DMA feeds both comp8 terms
    x8_d = nc.dram_tensor("x8c", [KT8, 128, 2, 2, T], f8, kind="ExternalInput").ap()
    w8_d = nc.dram_tensor("w8c", [KT8, 128, 2, 2 * 3 * CH], f8, kind="ExternalInput").ap()
    wp_d = nc.dram_tensor("wp", [2, 128, C], f16, kind="ExternalInput").ap()
    bqk_d = nc.dram_tensor("bqk", [128, 4], f32, kind="ExternalInput").ap()
    bv_d = nc.dram_tensor("bv", [1, CH], f32, kind="ExternalInput").ap()
    tri_d = nc.dram_tensor("tri", [128, 128], f16, kind="ExternalInput").ap()
    ident_d = nc.dram_tensor("ident", [128, 128], f16, kind="ExternalInput").ap()
    out_d = nc.dram_tensor("out_p", [T, C], f16, kind="ExternalOutput").ap()

    with tile.TileContext(nc) as tc:
        with (
            tc.tile_pool(name="p_w", bufs=1) as p_w,
            tc.tile_pool(name="p_x", bufs=1) as p_x,
            tc.tile_pool(name="p_qk", bufs=1) as p_qk,
            tc.tile_pool(name="p_v", bufs=1) as p_v,
            tc.tile_pool(name="p_y", bufs=1) as p_y,
            tc.tile_pool(name="p_pt", bufs=4) as p_pt,
            tc.tile_pool(name="p_yn", bufs=16) as p_yn,
            tc.tile_pool(name="p_st", bufs=3) as p_st,
            tc.tile_pool(name="ps_s", bufs=2, space="PSUM") as ps_s,
            tc.tile_pool(name="ps_y", bufs=1, space="PSUM") as ps_y,
            tc.tile_pool(name="ps_mm", bufs=2, space="PSUM") as ps_mm,
        ):
            # ---- persistent inputs -------------------------------------
            ident = p_w.tile([128, 128], f16, name="ident", tag="ident")
            tri = p_w.tile([128, 128], f16, name="tri", tag="tri")
            bqk = p_w.tile([128, 4], f32, name="bqk", tag="bqk")
            bvrow = p_w.tile([1, CH], f32, name="bvrow", tag="bvrow")
            bvb = p_w.tile([128, CH], f32, name="bvb", tag="bvb")
            w8c = [p_w.tile([128, 2, 2 * 3 * CH], f8, name=f"w8_{k}", tag=f"w8_{k}")
                   for k in range(KT8)]
            x8c = [p_x.tile([128, 2, 2, T], f8, name=f"x8_{k}", tag=f"x8_{k}")
                   for k in range(KT8)]
            wp = [p_w.tile([128, C], f16, name=f"wp{k}", tag=f"wp{k}")
                  for k in range(2)]
            # views into the [qk-main|qk-resid|v-main|v-resid] column layout
            w8qk = [w8c[k][:, :, 0:512] for k in range(KT8)]
            wr8qk = [w8c[k][:, :, 512:1024] for k in range(KT8)]
            w8v = [w8c[k][:, :, 1024:1280] for k in range(KT8)]
            wr8v = [w8c[k][:, :, 1280:1536] for k in range(KT8)]
            x8 = [x8c[k][:, :, 0, :] for k in range(KT8)]
            xr8 = [x8c[k][:, :, 1, :] for k in range(KT8)]

            # warmup source first (Pool memset, no deps) so PE can spin
            wsrc = p_w.tile([128, 128], f16, name="wsrc", tag="wsrc")
            nc.gpsimd.memset(wsrc, 0.5)
            # small consts via SWDGE (Pool) so HWDGE is free for the bulk ramp
            nc.gpsimd.dma_start(out=bqk, in_=bqk_d)
            nc.gpsimd.dma_start(out=bvrow, in_=bv_d)
            nc.gpsimd.dma_start(out=tri, in_=tri_d)
            nc.gpsimd.dma_start(out=ident, in_=ident_d)
            nc.gpsimd.partition_broadcast(bvb, bvrow[0:1, :])
            # bulk ramp: per k, first x t-quarter + comp8 qk weights (unlocks
            # the S side); v weights follow, then the remaining x quarters
            for k in range(KT8):
                nc.sync.dma_start(out=x8c[k][:, :, :, 0:512],
                                  in_=x8_d[k][:, :, :, 0:512])
                nc.sync.dma_start(out=w8c[k][:, :, 0:1024],
                                  in_=w8_d[k][:, :, 0:1024])
            for k in range(KT8):
                nc.sync.dma_start(out=w8c[k][:, :, 1024:1536],
                                  in_=w8_d[k][:, :, 1024:1536])
            for q in range(1, 4):
                for k in range(KT8):
                    nc.sync.dma_start(
                        out=x8c[k][:, :, :, 512 * q:512 * (q + 1)],
                        in_=x8_d[k][:, :, :, 512 * q:512 * (q + 1)])
            for k in range(2):
                nc.sync.dma_start(out=wp[k], in_=wp_d[k])

            # ---- persistent intermediates ------------------------------
            # qT/kT tile p: head pair p, heads (2p, 2p+1) on partitions 0:64/64:128
            qT = [p_qk.tile([128, T], f16, name=f"qT{p}", tag=f"qT{p}") for p in range(2)]
            kT = [p_qk.tile([128, T], f16, name=f"kT{p}", tag=f"kT{p}") for p in range(2)]
            # v tiles: [128 t, 4 heads, 65] -- col 0 of each head = 1.0 (denominator)
            v = [p_v.tile([128, 4, 65], f16, name=f"v{m}", tag=f"v{m}") for m in range(NT)]
            # normalized y^T per pair: [128 ch, T]
            ynT = [p_y.tile([128, T], f16, name=f"ynT{p}", tag=f"ynT{p}") for p in range(2)]

            # PE p-state warmup: cheap dependency-light matmuls
            warm = ps_mm.tile([128, 512], f32, name="warm", tag="mm")
            for _ in range(24):
                nc.tensor.matmul(warm[:, 0:128], lhsT=wsrc, rhs=wsrc,
                                 start=True, stop=True)

            GROUPS = ((x8, w8qk, w8v), (x8, wr8qk, wr8v), (xr8, w8qk, w8v))

            def qkv_mms(ps, mi, nj, glist):
                for g in glist:
                    xa, wb, _ = GROUPS[g]
                    for kk in range(KT8):
                        nc.tensor.matmul(
                            ps[:, 0:512],
                            lhsT=wb[kk][:, :, 128 * mi:128 * (mi + 1)],
                            rhs=xa[kk][:, :, 512 * nj:512 * (nj + 1)],
                            start=(g == 0 and kk == 0),
                            stop=(g == 2 and kk == KT8 - 1),
                            perf_mode=DR,
                        )

            def qkv_move(ps, mi, nj):
                dst = (qT if mi < 2 else kT)[mi % 2][:, 512 * nj:512 * (nj + 1)]
                nc.vector.tensor_scalar(dst, ps[:, 0:512], 1.0 / WS,
                                        bqk[:, mi:mi + 1], mult, add)

            def qkv_chunk(mi, nj):
                """q/k channels [128mi,128mi+128), t [512nj, 512nj+512)."""
                ps = ps_mm.tile([128, 512], f32, name="ps_qkv", tag="mm")
                qkv_mms(ps, mi, nj, (0, 1, 2))
                qkv_move(ps, mi, nj)

            def v_mms(ps, m, glist):
                for g in glist:
                    xa, _, wb = GROUPS[g]
                    for kk in range(KT8):
                        nc.tensor.matmul(
                            ps[:, 0:CH],
                            lhsT=xa[kk][:, :, 128 * m:128 * (m + 1)],
                            rhs=wb[kk],
                            start=(g == 0 and kk == 0),
                            stop=(g == 2 and kk == KT8 - 1),
                            perf_mode=DR,
                        )

            def v_move(ps, m):
                nc.vector.memset(v[m][:, :, 0:1], 1.0)
                nc.vector.scalar_tensor_tensor(
                    v[m][:, :, 1:65],
                    ps[:, 0:CH].rearrange("p (h c) -> p h c", h=4),
                    1.0 / WS,
                    bvb.rearrange("p (h c) -> p h c", h=4),
                    mult, add,
                )

            def v_chunk(m):
                """v rows [128m, 128m+128), all 4 heads."""
                ps = ps_mm.tile([128, 512], f32, name="ps_v", tag="mm")
                v_mms(ps, m, (0, 1, 2))
                v_move(ps, m)

            def attention(j, p, filler=None, depth=2, tail_hook=None):
                """q-chunk j (512 queries), head pair p (heads 2p, 2p+1).
                AV is software-pipelined `depth` chunks behind S/exp so the
                PE stream never parks on the exp it just requested."""
                ni = 4 * j + 4
                yp = [ps_y.tile([128, 4, 65], f32, name=f"y{h}", tag=f"y{h}")
                      for h in range(2)]
                pts = {}

                def av(i):
                    rr = i - 4 * j
                    pt = pts.pop(i)
                    for h in range(2):
                        for tt in range(max(0, rr), 4):
                            nc.tensor.matmul(
                                yp[h][:, tt, 0:65],
                                lhsT=pt[:, h, 128 * tt:128 * (tt + 1)],
                                rhs=v[i][:, 2 * p + h, :],
                                start=(i == 0 and tt == 0),
                                stop=(i == 4 * j + tt),
                                skip_group_check=True,
                            )

                for i in range(ni):
                    rr = i - 4 * j
                    W0 = 128 * rr if rr > 0 else 0
                    s2 = ps_s.tile([128, 2, 512], f32, name="s2", tag="s")
                    for h in range(2):
                        nc.tensor.matmul(
                            s2[:, h, W0:512],
                            lhsT=kT[p][64 * h:64 * h + 64, 128 * i:128 * (i + 1)],
                            rhs=qT[p][64 * h:64 * h + 64, 512 * j + W0:512 * (j + 1)],
                            start=True, stop=True,
                        )
                    pt = p_pt.tile([128, 2, 512], f16, name="pt", tag="pt")
                    nc.scalar.activation(pt[:, :, W0:512], s2[:, :, W0:512], Exp)
                    pts[i] = pt
                    if rr >= 0:
                        for h in range(2):
                            nc.vector.tensor_tensor(
                                pt[:, h, W0:W0 + 128], pt[:, h, W0:W0 + 128],
                                tri, mult)
                    if i >= depth:
                        av(i - depth)
                    if filler is not None:
                        filler()
                for i in range(ni - depth, ni):
                    av(i)
                # normalize (denominator = col 0) now; defer the PE transpose
                # + ynT move so the next round's S-matmuls aren't parked
                # behind this round's DVE queue.
                deferred = []
                # ISA TensorScalar has no divide: one reciprocal per head
                # (denominator col 0, all 4 q-subtiles), then scalar-multiply
                rc = p_yn.tile([128, 2, 4], f32, name="rc", tag="rc", bufs=4)
                for h in range(2):
                    nc.vector.reciprocal(
                        rc[:, h, :],
                        yp[h][:, :, 0:1].rearrange("p s o -> p (s o)"))
                for tt in range(4):
                    ynst = p_yn.tile([128, 128], f16, name="ynst", tag="yn")
                    for h in range(2):
                        nc.vector.tensor_scalar(
                            ynst[:, 64 * h:64 * h + 64],
                            yp[h][:, tt, 1:65], rc[:, h, tt:tt + 1], None, mult)

                    def transpose_move(tt=tt, ynst=ynst):
                        tp = ps_mm.tile([128, 128], f16, name="tp", tag="mm")
                        nc.tensor.transpose(tp, ynst, ident)
                        nc.vector.tensor_copy(
                            ynT[p][:, 512 * j + 128 * tt:
                                   512 * j + 128 * (tt + 1)], tp)
                    deferred.append(transpose_move)
                if tail_hook is not None:
                    # pipeline: transposes run ahead of the projs needing them
                    deferred[0]()
                    deferred[1]()
                    for tt in range(4):
                        if tt + 2 < 4:
                            deferred[tt + 2]()
                        tail_hook(tt)
                    return []
                return deferred

            def proj(m, tail=False):
                """output rows [128m, 128m+128): 2 c-halves into one staging tile.
                Steady state: staging on Pool, DMA via SWDGE (keeps DVE/HWDGE
                free). Tail: parallel DVE+ACT staging, half-DMAs via HWDGE."""
                st = p_st.tile([128, 1024], f16, name="st_pr", tag="st")
                for u in range(2):
                    ps = ps_mm.tile([128, 512], f32, name="ps_pr", tag="mm")
                    for kk in range(2):
                        nc.tensor.matmul(
                            ps[:, 0:512],
                            lhsT=ynT[kk][:, 128 * m:128 * (m + 1)],
                            rhs=wp[kk][:, 512 * u:512 * (u + 1)],
                            start=(kk == 0), stop=(kk == 1),
                        )
                    stu = st[:, 512 * u:512 * (u + 1)]
                    if tail:
                        if u == 0:
                            nc.vector.tensor_copy(stu, ps[:, 0:512])
                        else:
                            nc.scalar.copy(stu, ps[:, 0:512])
                        nc.sync.dma_start(
                            out=out_d[128 * m:128 * (m + 1),
                                      512 * u:512 * (u + 1)],
                            in_=stu)
                    else:
                        nc.vector.tensor_copy(stu, ps[:, 0:512])
                if not tail:
                    nc.gpsimd.dma_start(
                        out=out_d[128 * m:128 * (m + 1), :], in_=st)

            # ---- emission order (scheduling priority) -------------------
            # ramp: wave 0, first two chunks split so the fp8 main group runs
            # as soon as w8/x8 land; residuals follow when wr8/xr8 arrive.
            ps_a = ps_mm.tile([128, 512], f32, name="ps_qkv", tag="mm")
            qkv_mms(ps_a, 0, 0, (0,))
            ps_b = ps_mm.tile([128, 512], f32, name="ps_qkv", tag="mm")
            qkv_mms(ps_b, 2, 0, (0,))
            qkv_mms(ps_a, 0, 0, (1, 2))
            qkv_move(ps_a, 0, 0)
            qkv_mms(ps_b, 2, 0, (1, 2))
            qkv_move(ps_b, 2, 0)
            for mi in (1, 3):
                qkv_chunk(mi, 0)
            # v(0..3) ride as the first fillers of attention(0,0): their
            # x/w data lands after the qk stream, and j=0 runs AV depth-4
            # so no AV precedes them.

            # waves: qkv/v chunks -- must fully emit before the attention
            # round that reads them (drained at each j boundary).
            # ordered: transposes + projs -- free to span boundaries.
            waves = []
            ordered = []
            budget = [None]  # per-round cap on consumed fillers

            def filler():
                if budget[0] is not None:
                    if budget[0] <= 0:
                        return
                    budget[0] -= 1
                if waves:
                    waves.pop(0)()
                elif ordered:
                    ordered.pop(0)()

            for m in range(4):
                waves.append(lambda m=m: v_chunk(m))
            for j in range(NJ):
                if j < 3:
                    nxt = j + 1
                    for mi in (0, 2):
                        waves.append(lambda mi=mi, nj=nxt: qkv_chunk(mi, nj))
                    # v rows for the last q-chunk are only read late in its
                    # own i-loop; defer them there to feed the ACT-paced j=3.
                    for m in range(4 * nxt, 4 * nxt + (4 if nxt < 3 else 0)):
                        waves.append(lambda m=m: v_chunk(m))
                    for mi in (1, 3):
                        waves.append(lambda mi=mi, nj=nxt: qkv_chunk(mi, nj))
                # earlier t-blocks' projections, kept late to feed PE while
                # ACT drains the (larger) late-j exp queue
                if j == 2:
                    for m in range(0, 4):
                        ordered.append(lambda m=m: proj(m))
                elif j == 3:
                    for m in range(12, 16):
                        waves.insert(0, lambda m=m: v_chunk(m))
                    for m in range(4, 12):
                        ordered.append(lambda m=m: proj(m))
                for p in range(2):
                    budget[0] = 8 if (j, p) == (3, 0) else None
                    if (j, p) == (3, 1):
                        def tail_hook(tt):
                            while ordered:
                                ordered.pop(0)()
                            proj(12 + tt, tail=True)
                        attention(j, p, filler, tail_hook=tail_hook)
                    else:
                        deferred = attention(j, p, filler,
                                             depth=(4 if j == 0 else 2))
                        ordered.extend(deferred)
                while waves:
                    waves.pop(0)()
            budget[0] = None

    nc.compile()
    return nc


def _host_inputs(x, W_attn, b_attn, W_proj):
    """Build the 8 per-core input maps (numpy only)."""
    import ml_dtypes
    f8 = ml_dtypes.float8_e4m3

    x = np.asarray(x, dtype=np.float32)
    W_attn = np.asarray(W_attn, dtype=np.float32)
    b_attn = np.asarray(b_attn, dtype=np.float32)
    W_proj = np.asarray(W_proj, dtype=np.float32)

    # strict causal 0/1 mask for the 128x128 diagonal blocks: valid iff c >= k
    kl = np.arange(128)
    tri = (kl[None, :] >= kl[:, None]).astype(np.float16)
    ident = np.eye(128, dtype=np.float16)

    def pack8(a):
        """[C, N] -> fp8 main/residual tiles [KT8, 128, 2, N] each."""
        a8 = a.astype(f8)
        ar8 = (a - a8.astype(np.float32)).astype(f8)
        def t(z):
            return z.reshape(KT8, 2, 128, a.shape[1]).transpose(0, 2, 1, 3)
        return t(a8), t(ar8)

    in_maps = []
    for c in range(NCORES):
        b, g = divmod(c, 4)
        sl = slice(CH * g, CH * (g + 1))
        wq = W_attn[:, 0 * C:1 * C][:, sl] * SCALE
        wk = W_attn[:, 1 * C:2 * C][:, sl]
        wv = W_attn[:, 2 * C:3 * C][:, sl]
        bq = b_attn[0 * C:1 * C][sl] * SCALE
        bk = b_attn[1 * C:2 * C][sl]
        bv = b_attn[2 * C:3 * C][sl]
        bqk = np.stack([bq[0:128], bq[128:256], bk[0:128], bk[128:256]], axis=1)
        wfull = np.concatenate([wq, wk, wv], axis=1) * WS     # [1024, 768]
        w8, wr8 = pack8(wfull)
        xT = np.ascontiguousarray(x[b].T)                     # [1024, 2048]
        x8, xr8 = pack8(xT)
        # columns: [qk-main | qk-resid | v-main | v-resid]
        w8c = np.ascontiguousarray(np.concatenate(
            [w8[..., 0:512], wr8[..., 0:512],
             w8[..., 512:768], wr8[..., 512:768]], axis=3))
        x8c = np.ascontiguousarray(np.stack([x8, xr8], axis=3))
        in_maps.append({
            "x8c": x8c, "w8c": w8c,
            "wp": np.ascontiguousarray(
                W_proj[sl, :].reshape(2, 128, C).astype(np.float16)),
            "bqk": np.ascontiguousarray(bqk),
            "bv": np.ascontiguousarray(bv[None, :]),
            "tri": tri, "ident": ident,
        })
    return in_maps


def kernel(x, W_attn, b_attn, W_proj, b_proj, _want_results=None):
    global _COMPILED
    from concourse.bass_utils import run_bass_kernel_spmd

    if _COMPILED is None:
        _COMPILED = _build()
    nc = _COMPILED

    in_maps = _host_inputs(x, W_attn, b_attn, W_proj)
    kw = dict(_want_results or {})
    res = run_bass_kernel_spmd(nc, in_maps, core_ids=list(range(NCORES)), **kw)
    if _want_results is not None:
        kernel.last_results = res

    out = np.zeros((B, T, C), dtype=np.float32)
    for c in range(NCORES):
        out[c // 4] += res.results[c]["out_p"].astype(np.float32)
    out += np.asarray(b_proj, dtype=np.float32)[None, None, :]
    return out


# revision 82
# speedup vs baseline: 1.4188x; 1.0463x over previous
"""Causal self-attention (B=2, T=2048, C=1024, H=16) on 8 TRN2 NeuronCores.

Sharding: core c -> batch b = c//4, head-group g = c%4 (4 heads = 256 channels).
Each core computes its 4 heads end-to-end and a partial projection
(y_local @ W_proj[256g:256g+256, :]); the host sums the 4 partials per batch.

v2 dataflow (cost-model-driven):
  - QKV/V matmuls in fp8e4m3 DoubleRow with two-term compensation:
    (x8+xr8)@(w8+wr8) dropping xr8@wr8 -> 12 DR passes per chunk at
    0.5 cyc/col (0.75x the fp32r cost), weights pre-scaled x64 on host
    (descaled in the psum->sbuf move) to clear fp8's subnormal floor.
  - S^T = k_h^T q_h per head in f16, exact 128-granular causal windows.
    Both heads of a pair share one [128,2,512] psum tile; one fused exp
    per chunk on ACT.
  - Causal masking via 0/1 f16 multiply on DVE (2x mode) after exp --
    no mask matmuls on PE.
  - AV in natural [q,d] layout: lhsT = pt q-slices (stationary loads are
    free in the cost model), rhs = [1|v_h] -> 65-col matmuls, half the
    transposed-layout cost. Softmax denominator rides column 0; the
    normalize is a per-partition tensor_scalar divide (no broadcasts).
  - Normalized y transposed back via PE is_transpose (f16), then f16 proj.
  - Output f16; host upcasts, sums partials, adds b_proj.

Scheduling: engines run their streams in emission order; qkv/v waves and
proj work are interleaved into the ACT-paced attention chunks as fillers.
Outputs DMA via gpsimd SWDGE to keep HWDGE free for the input stream.
"""

import numpy as np

B, T, C = 2, 2048, 1024
H, HD = 16, 64
NCORES = 8
HEADS_PER_CORE = 4          # 2 pairs
CH = HEADS_PER_CORE * HD    # 256 channels per core
KT8 = 4                     # fp8 contraction pair-tiles (K=256 each)
NT = T // 128               # 16 t tiles
NJ = T // 512               # 4 query chunks
SCALE = 1.0 / np.sqrt(HD)
WS = 64.0                   # host-side weight pre-scale for fp8 range

_COMPILED = None


def _build():
    import concourse.bass as bass
    import concourse.bacc as bacc
    import concourse.mybir as mybir
    import concourse.tile as tile

    f32 = mybir.dt.float32
    f16 = mybir.dt.float16
    f8 = mybir.dt.float8e4
    DR = mybir.MatmulPerfMode.DoubleRow
    Exp = mybir.ActivationFunctionType.Exp
    mult = mybir.AluOpType.mult
    add = mybir.AluOpType.add
    div = mybir.AluOpType.divide

    nc = bacc.Bacc("TRN2", target_bir_lowering=False, debug=False)

    # combined main||residual fp8 tensors: one DMA feeds both comp8 terms
    x8_d = nc.dram_tensor("x8c", [KT8, 128, 2, 2, T], f8, kind="ExternalInput").ap()
    w8_d = nc.dram_tensor("w8c", [KT8, 128, 2, 2 * 3 * CH], f8, kind="ExternalInput").ap()
    wp_d = nc.dram_tensor("wp", [2, 128, C], f16, kind="ExternalInput").ap()
    bqk_d = nc.dram_tensor("bqk", [128, 4], f32, kind="ExternalInput").ap()
    bv_d = nc.dram_tensor("bv", [1, CH], f32, kind="ExternalInput").ap()
    tri_d = nc.dram_tensor("tri", [128, 128], f16, kind="ExternalInput").ap()
    ident_d = nc.dram_tensor("ident", [128, 128], f16, kind="ExternalInput").ap()
    out_d = nc.dram_tensor("out_p", [T, C], f16, kind="ExternalOutput").ap()

    with tile.TileContext(nc) as tc:
        with (
            tc.tile_pool(name="p_w", bufs=1) as p_w,
            tc.tile_pool(name="p_x", bufs=1) as p_x,
            tc.tile_pool(name="p_qk", bufs=1) as p_qk,
            tc.tile_pool(name="p_v", bufs=1) as p_v,
            tc.tile_pool(name="p_y", bufs=1) as p_y,
            tc.tile_pool(name="p_pt", bufs=12) as p_pt,
            tc.tile_pool(name="p_yn", bufs=16) as p_yn,
            tc.tile_pool(name="p_st", bufs=4) as p_st,
            tc.tile_pool(name="ps_s", bufs=2, space="PSUM") as ps_s,
            tc.tile_pool(name="ps_y", bufs=1, space="PSUM") as ps_y,
            tc.tile_pool(name="ps_mm", bufs=2, space="PSUM") as ps_mm,
        ):
            # ---- persistent inputs -------------------------------------
            ident = p_w.tile([128, 128], f16, name="ident", tag="ident")
            tri = p_w.tile([128, 128], f16, name="tri", tag="tri")
            bqk = p_w.tile([128, 4], f32, name="bqk", tag="bqk")
            bvrow = p_w.tile([1, CH], f32, name="bvrow", tag="bvrow")
            bvb = p_w.tile([128, CH], f32, name="bvb", tag="bvb")
            w8c = [p_w.tile([128, 2, 2 * 3 * CH], f8, name=f"w8_{k}", tag=f"w8_{k}")
                   for k in range(KT8)]
            x8c = [p_x.tile([128, 2, 2, T], f8, name=f"x8_{k}", tag=f"x8_{k}")
                   for k in range(KT8)]
            wp = [p_w.tile([128, C], f16, name=f"wp{k}", tag=f"wp{k}")
                  for k in range(2)]
            # views into the [qk-main|qk-resid|v-main|v-resid] column layout
            w8qk = [w8c[k][:, :, 0:512] for k in range(KT8)]
            wr8qk = [w8c[k][:, :, 512:1024] for k in range(KT8)]
            w8v = [w8c[k][:, :, 1024:1280] for k in range(KT8)]
            wr8v = [w8c[k][:, :, 1280:1536] for k in range(KT8)]
            x8 = [x8c[k][:, :, 0, :] for k in range(KT8)]
            xr8 = [x8c[k][:, :, 1, :] for k in range(KT8)]

            # warmup source first (Pool memset, no deps) so PE can spin
            wsrc = p_w.tile([128, 128], f16, name="wsrc", tag="wsrc")
            nc.gpsimd.memset(wsrc, 0.5)
            # small consts via SWDGE (Pool) so HWDGE is free for the bulk ramp
            nc.gpsimd.dma_start(out=bqk, in_=bqk_d)
            nc.gpsimd.dma_start(out=bvrow, in_=bv_d)
            nc.gpsimd.dma_start(out=tri, in_=tri_d)
            nc.gpsimd.dma_start(out=ident, in_=ident_d)
            nc.gpsimd.partition_broadcast(bvb, bvrow[0:1, :])
            # bulk ramp: per k, first x t-quarter + comp8 qk weights (unlocks
            # the S side); v weights follow, then the remaining x quarters
            for k in range(KT8):
                nc.sync.dma_start(out=x8c[k][:, :, :, 0:512],
                                  in_=x8_d[k][:, :, :, 0:512])
                nc.sync.dma_start(out=w8c[k][:, :, 0:1024],
                                  in_=w8_d[k][:, :, 0:1024])
            for k in range(KT8):
                nc.sync.dma_start(out=w8c[k][:, :, 1024:1536],
                                  in_=w8_d[k][:, :, 1024:1536])
            for q in (1, 2, 3):
                for k in range(KT8):
                    nc.sync.dma_start(
                        out=x8c[k][:, :, :, 512 * q:512 * (q + 1)],
                        in_=x8_d[k][:, :, :, 512 * q:512 * (q + 1)])
            for k in range(2):
                nc.sync.dma_start(out=wp[k], in_=wp_d[k])

            # ---- persistent intermediates ------------------------------
            # qT/kT tile p: head pair p, heads (2p, 2p+1) on partitions 0:64/64:128
            qT = [p_qk.tile([128, T], f16, name=f"qT{p}", tag=f"qT{p}") for p in range(2)]
            kT = [p_qk.tile([128, T], f16, name=f"kT{p}", tag=f"kT{p}") for p in range(2)]
            # v tiles: [128 t, 4 heads, 65] -- col 0 of each head = 1.0 (denominator)
            v = [p_v.tile([128, 4, 65], f16, name=f"v{m}", tag=f"v{m}") for m in range(NT)]
            # normalized y^T per pair: [128 ch, T]
            ynT = [p_y.tile([128, T], f16, name=f"ynT{p}", tag=f"ynT{p}") for p in range(2)]

            # PE p-state warmup: cheap dependency-light matmuls
            warm = ps_mm.tile([128, 512], f32, name="warm", tag="mm")
            for _ in range(32):
                nc.tensor.matmul(warm[:, 0:128], lhsT=wsrc, rhs=wsrc,
                                 start=True, stop=True)

            GROUPS = ((x8, w8qk, w8v), (x8, wr8qk, wr8v), (xr8, w8qk, w8v))

            def qkv_mms(ps, mi, nj, glist):
                for g in glist:
                    xa, wb, _ = GROUPS[g]
                    for kk in range(KT8):
                        nc.tensor.matmul(
                            ps[:, 0:512],
                            lhsT=wb[kk][:, :, 128 * mi:128 * (mi + 1)],
                            rhs=xa[kk][:, :, 512 * nj:512 * (nj + 1)],
                            start=(g == 0 and kk == 0),
                            stop=(g == 2 and kk == KT8 - 1),
                            perf_mode=DR,
                        )

            def qkv_move(ps, mi, nj):
                dst = (qT if mi < 2 else kT)[mi % 2][:, 512 * nj:512 * (nj + 1)]
                nc.vector.tensor_scalar(dst, ps[:, 0:512], 1.0 / WS,
                                        bqk[:, mi:mi + 1], mult, add)

            def qkv_chunk(mi, nj):
                """q/k channels [128mi,128mi+128), t [512nj, 512nj+512)."""
                ps = ps_mm.tile([128, 512], f32, name="ps_qkv", tag="mm")
                qkv_mms(ps, mi, nj, (0, 1, 2))
                qkv_move(ps, mi, nj)

            def qkv_pieces(mi, nj):
                """qkv chunk as 3 filler pieces (~0.43us each) so per-slot
                PE filler mass matches the exp-pacing deficit."""
                st = {}

                def p0():
                    st["ps"] = ps_mm.tile([128, 512], f32, name="ps_qkv",
                                          tag="mm")
                    qkv_mms(st["ps"], mi, nj, (0,))
                    qkv_mms(st["ps"], mi, nj, (1,))
                return [p0,
                        lambda: (qkv_mms(st["ps"], mi, nj, (2,)),
                                 qkv_move(st["ps"], mi, nj))]

            def v_pieces(m):
                st = {}

                def p0():
                    st["ps"] = ps_mm.tile([128, 512], f32, name="ps_v",
                                          tag="mm")
                    v_mms(st["ps"], m, (0, 1))
                return [p0,
                        lambda: (v_mms(st["ps"], m, (2,)), v_move(st["ps"], m))]

            def proj_pieces(m):
                st = {"st": None}

                def pu(u):
                    if u == 0:
                        st["st"] = p_st.tile([128, 1024], f16, name="st_pr",
                                             tag="st")
                    ps = ps_mm.tile([128, 512], f32, name="ps_pr", tag="mm")
                    for kk in range(2):
                        nc.tensor.matmul(
                            ps[:, 0:512],
                            lhsT=ynT[kk][:, 128 * m:128 * (m + 1)],
                            rhs=wp[kk][:, 512 * u:512 * (u + 1)],
                            start=(kk == 0), stop=(kk == 1),
                        )
                    nc.vector.tensor_copy(
                        st["st"][:, 512 * u:512 * (u + 1)], ps[:, 0:512])
                    if u == 1:
                        nc.sync.dma_start(
                            out=out_d[128 * m:128 * (m + 1), :], in_=st["st"])
                return [lambda: pu(0), lambda: pu(1)]

            def v_mms(ps, m, glist):
                for g in glist:
                    xa, _, wb = GROUPS[g]
                    for kk in range(KT8):
                        nc.tensor.matmul(
                            ps[:, 0:CH],
                            lhsT=xa[kk][:, :, 128 * m:128 * (m + 1)],
                            rhs=wb[kk],
                            start=(g == 0 and kk == 0),
                            stop=(g == 2 and kk == KT8 - 1),
                            perf_mode=DR,
                        )

            def v_move(ps, m):
                nc.vector.memset(v[m][:, :, 0:1], 1.0)
                nc.vector.scalar_tensor_tensor(
                    v[m][:, :, 1:65],
                    ps[:, 0:CH].rearrange("p (h c) -> p h c", h=4),
                    1.0 / WS,
                    bvb.rearrange("p (h c) -> p h c", h=4),
                    mult, add,
                )

            def v_chunk(m):
                """v rows [128m, 128m+128), all 4 heads."""
                ps = ps_mm.tile([128, 512], f32, name="ps_v", tag="mm")
                v_mms(ps, m, (0, 1, 2))
                v_move(ps, m)

            prerolled = {}

            def pool_exp(out, in_):
                """exp on the gpsimd engine: parallel pt-production queue."""
                eng = nc.gpsimd
                imm = lambda val: mybir.ImmediateValue(dtype=f32, value=val)
                eng.add_instruction(mybir.InstActivation(
                    name=eng.bass.get_next_instruction_name(),
                    func=Exp,
                    ins=[eng.lower_ap(in_), imm(0.0), imm(1.0), imm(0.0)],
                    outs=[eng.lower_ap(out)],
                ))

            def s_exp_chunk(j, p, i):
                """S matmuls + fused exp (+ diag tri-mask) for one chunk."""
                rr = i - 4 * j
                W0 = 128 * rr if rr > 0 else 0
                s2 = ps_s.tile([128, 2, 512], f32, name="s2", tag="s")
                for h in range(2):
                    nc.tensor.matmul(
                        s2[:, h, W0:512],
                        lhsT=kT[p][64 * h:64 * h + 64, 128 * i:128 * (i + 1)],
                        rhs=qT[p][64 * h:64 * h + 64, 512 * j + W0:512 * (j + 1)],
                        start=True, stop=True,
                    )
                pt = p_pt.tile([128, 2, 512], f16, name="pt", tag="pt")
                if False:
                    pool_exp(pt[:, :, W0:512], s2[:, :, W0:512])
                else:
                    nc.scalar.activation(pt[:, :, W0:512], s2[:, :, W0:512], Exp)
                if rr >= 0:
                    for h in range(2):
                        nc.vector.tensor_tensor(
                            pt[:, h, W0:W0 + 128], pt[:, h, W0:W0 + 128],
                            tri, mult)
                return pt

            def attention(j, p, filler=None, depth=2, tail_hook=None,
                          next_jp=None, preroll_drain=None):
                """q-chunk j (512 queries), head pair p (heads 2p, 2p+1).
                AV is software-pipelined `depth` chunks behind S/exp so the
                PE stream never parks on the exp it just requested. The next
                round's first S/exp chunk is pre-rolled before the AV tail to
                hide the s2-pool rotation wait at the round boundary."""
                ni = 4 * j + 4
                yp = [ps_y.tile([128, 4, 65], f32, name=f"y{h}", tag=f"y{h}")
                      for h in range(2)]
                pts = prerolled.pop((j, p), {})

                deferred = []

                def av(i):
                    rr = i - 4 * j
                    pt = pts.pop(i)
                    for h in range(2):
                        for tt in range(max(0, rr), 4):
                            nc.tensor.matmul(
                                yp[h][:, tt, 0:65],
                                lhsT=pt[:, h, 128 * tt:128 * (tt + 1)],
                                rhs=v[i][:, 2 * p + h, :],
                                start=(i == 0 and tt == 0),
                                stop=(i == 4 * j + tt),
                                skip_group_check=True,
                            )
                    # q-subtile tt's accumulation closed at chunk 4j+tt:
                    # stream its normalize (+ tail work) immediately
                    tt = i - 4 * j
                    if tt >= 0:
                        norm(tt)

                def norm(tt):
                    # ISA TensorScalar has no divide: per-head reciprocal of
                    # the denominator (col 0), then scalar-multiply
                    rc = p_yn.tile([128, 2], f32, name="rc", tag="rc", bufs=8)
                    ynst = p_yn.tile([128, 128], f16, name="ynst", tag="yn")
                    for h in range(2):
                        nc.vector.reciprocal(rc[:, h:h + 1], yp[h][:, tt, 0:1])
                        nc.vector.tensor_scalar(
                            ynst[:, 64 * h:64 * h + 64],
                            yp[h][:, tt, 1:65], rc[:, h:h + 1], None, mult)

                    def transpose_move(tt=tt, ynst=ynst):
                        tp = ps_mm.tile([128, 128], f16, name="tp", tag="mm")
                        nc.tensor.transpose(tp, ynst, ident)
                        nc.vector.tensor_copy(
                            ynT[p][:, 512 * j + 128 * tt:
                                   512 * j + 128 * (tt + 1)], tp)
                    if tail_hook is not None:
                        transpose_move()
                        tail_hook(tt)
                    else:
                        deferred.append(transpose_move)

                for i in range(ni):
                    if i not in pts:
                        pts[i] = s_exp_chunk(j, p, i)
                    if i >= depth:
                        av(i - depth)
                    if filler is not None:
                        filler()
                if next_jp is not None:
                    # the next round reads qT/kT written by wave fillers --
                    # force the relevant ones out first
                    preroll_drain(next_jp)
                    prerolled[next_jp] = {0: s_exp_chunk(*next_jp, 0)}
                for i in range(ni - depth, ni):
                    av(i)
                return deferred

            def proj(m, tail=False):
                """output rows [128m, 128m+128): 2 c-halves into one staging tile.
                Steady state: staging on Pool, DMA via SWDGE (keeps DVE/HWDGE
                free). Tail: parallel DVE+ACT staging, half-DMAs via HWDGE."""
                st = p_st.tile([128, 1024], f16, name="st_pr", tag="st")
                for u in range(2):
                    if tail and u == 1:
                        # S pool is idle in the tail; avoids mm-slot waits
                        ps = ps_s.tile([128, 512], f32, name="ps_prs", tag="s")
                    else:
                        ps = ps_mm.tile([128, 512], f32, name="ps_pr", tag="mm")
                    for kk in range(2):
                        nc.tensor.matmul(
                            ps[:, 0:512],
                            lhsT=ynT[kk][:, 128 * m:128 * (m + 1)],
                            rhs=wp[kk][:, 512 * u:512 * (u + 1)],
                            start=(kk == 0), stop=(kk == 1),
                        )
                    stu = st[:, 512 * u:512 * (u + 1)]
                    if tail and u == 1:
                        nc.scalar.copy(stu, ps[:, 0:512])
                    else:
                        nc.vector.tensor_copy(stu, ps[:, 0:512])
                eng = nc.gpsimd if (tail and m % 2 == 0) else nc.sync
                eng.dma_start(out=out_d[128 * m:128 * (m + 1), :], in_=st)

            # ---- emission order (scheduling priority) -------------------
            # ramp: wave 0, first two chunks split so the fp8 main group runs
            # as soon as w8/x8 land; residuals follow when wr8/xr8 arrive.
            ps_a = ps_mm.tile([128, 512], f32, name="ps_qkv", tag="mm")
            qkv_mms(ps_a, 0, 0, (0,))
            ps_b = ps_mm.tile([128, 512], f32, name="ps_qkv", tag="mm")
            qkv_mms(ps_b, 2, 0, (0,))
            qkv_mms(ps_a, 0, 0, (1, 2))
            qkv_move(ps_a, 0, 0)
            qkv_mms(ps_b, 2, 0, (1, 2))
            qkv_move(ps_b, 2, 0)
            for mi in (1, 3):
                qkv_chunk(mi, 0)
            # v(0..3) ride as the first fillers of attention(0,0): their
            # x/w data lands after the qk stream, and j=0 runs AV depth-4
            # so no AV precedes them.

            # waves: pair-0 qkv of the next j -- must emit before attention
            # (j+1, 0) (drained at the j boundary / cross-j preroll).
            # waves_late: pair-1 qkv -- only read by (j+1, 1); they fill
            # (j+1, 0)'s ACT-paced slots and drain at the (j+1,1) preroll.
            # v chunks of wave w front-fill (w, 0): av(m) runs late in its
            # own round. ordered: transposes + projs -- span boundaries.
            waves = []
            waves_late = []
            ordered = []
            budget = [None]  # per-round cap on consumed `ordered` fillers

            def filler():
                if waves:
                    waves.pop(0)()
                elif waves_late:
                    waves_late.pop(0)()
                elif ordered:
                    if budget[0] is not None:
                        if budget[0] <= 0:
                            return
                        budget[0] -= 1
                    ordered.pop(0)()

            def drain(lst):
                while lst:
                    lst.pop(0)()

            vfront = {w: [lambda m=m: v_chunk(m)
                          for m in range(4 * w, 4 * w + 4)]
                      for w in range(4)}

            def preroll_drain(next_jp):
                if next_jp[1] == 1:
                    drain(waves_late)
                else:
                    drain(waves)

            # phase order front-loads the big j=3 round right after its data
            # lands, so ACT saturates early and the later (smaller) rounds
            # swim in proj/wave filler mass.
            # causality: attention(j) reads kT columns of ALL waves <= j, so
            # rounds must run in j order.
            for j in range(NJ):
                if j < 3:
                    nxt_w = j + 1
                    for mi in (0, 2):
                        waves.append(lambda mi=mi, nj=nxt_w: qkv_chunk(mi, nj))
                    for mi in (1, 3):
                        waves_late.append(lambda mi=mi, nj=nxt_w: qkv_chunk(mi, nj))
                # earlier t-blocks' projections, kept late to feed PE while
                # ACT drains the (larger) late-j exp queue
                if j == 2:
                    for m in range(0, 4):
                        ordered.append(lambda m=m: proj(m))
                elif j == 3:
                    for m in range(4, 12):
                        ordered.append(lambda m=m: proj(m))
                for p in range(2):
                    if p == 0:
                        waves[0:0] = vfront.pop(j)
                    budget[0] = 4 if (j, p) == (3, 0) else None
                    nxt = (j, 1) if p == 0 else ((j + 1, 0) if j < 3 else None)
                    if (j, p) == (3, 1):
                        def tail_hook(tt):
                            drain(ordered)
                            proj(12 + tt, tail=True)
                        attention(j, p, filler, tail_hook=tail_hook)
                    else:
                        deferred = attention(j, p, filler,
                                             depth=(4 if j == 0 else 8),
                                             next_jp=nxt,
                                             preroll_drain=preroll_drain)
                        ordered.extend(deferred)
                drain(waves)

    nc.compile()
    return nc


def _host_inputs(x, W_attn, b_attn, W_proj):
    """Build the 8 per-core input maps (numpy only)."""
    import ml_dtypes
    f8 = ml_dtypes.float8_e4m3

    x = np.asarray(x, dtype=np.float32)
    W_attn = np.asarray(W_attn, dtype=np.float32)
    b_attn = np.asarray(b_attn, dtype=np.float32)
    W_proj = np.asarray(W_proj, dtype=np.float32)

    # strict causal 0/1 mask for the 128x128 diagonal blocks: valid iff c >= k
    kl = np.arange(128)
    tri = (kl[None, :] >= kl[:, None]).astype(np.float16)
    ident = np.eye(128, dtype=np.float16)

    def pack8(a):
        """[C, N] -> fp8 main/residual tiles [KT8, 128, 2, N] each."""
        a8 = a.astype(f8)
        ar8 = (a - a8.astype(np.float32)).astype(f8)
        def t(z):
            return z.reshape(KT8, 2, 128, a.shape[1]).transpose(0, 2, 1, 3)
        return t(a8), t(ar8)

    in_maps = []
    for c in range(NCORES):
        b, g = divmod(c, 4)
        sl = slice(CH * g, CH * (g + 1))
        wq = W_attn[:, 0 * C:1 * C][:, sl] * SCALE
        wk = W_attn[:, 1 * C:2 * C][:, sl]
        wv = W_attn[:, 2 * C:3 * C][:, sl]
        bq = b_attn[0 * C:1 * C][sl] * SCALE
        bk = b_attn[1 * C:2 * C][sl]
        bv = b_attn[2 * C:3 * C][sl]
        bqk = np.stack([bq[0:128], bq[128:256], bk[0:128], bk[128:256]], axis=1)
        wfull = np.concatenate([wq, wk, wv], axis=1) * WS     # [1024, 768]
        w8, wr8 = pack8(wfull)
        xT = np.ascontiguousarray(x[b].T)                     # [1024, 2048]
        x8, xr8 = pack8(xT)
        # columns: [qk-main | qk-resid | v-main | v-resid]
        w8c = np.ascontiguousarray(np.concatenate(
            [w8[..., 0:512], wr8[..., 0:512],
             w8[..., 512:768], wr8[..., 512:768]], axis=3))
        x8c = np.ascontiguousarray(np.stack([x8, xr8], axis=3))
        in_maps.append({
            "x8c": x8c, "w8c": w8c,
            "wp": np.ascontiguousarray(
                W_proj[sl, :].reshape(2, 128, C).astype(np.float16)),
            "bqk": np.ascontiguousarray(bqk),
            "bv": np.ascontiguousarray(bv[None, :]),
            "tri": tri, "ident": ident,
        })
    return in_maps


def kernel(x, W_attn, b_attn, W_proj, b_proj, _want_results=None):
    global _COMPILED
    from concourse.bass_utils import run_bass_kernel_spmd

    if _COMPILED is None:
        _COMPILED = _build()
    nc = _COMPILED

    in_maps = _host_inputs(x, W_attn, b_attn, W_proj)
    kw = dict(_want_results or {})
    res = run_bass_kernel_spmd(nc, in_maps, core_ids=list(range(NCORES)), **kw)
    if _want_results is not None:
        kernel.last_results = res

    out = np.zeros((B, T, C), dtype=np.float32)
    for c in range(NCORES):
        out[c // 4] += res.results[c]["out_p"].astype(np.float32)
    out += np.asarray(b_proj, dtype=np.float32)[None, None, :]
    return out
